# revision 7
# baseline (speedup 1.0000x reference)
"""ABCLinear distributed Bass kernel for 8 TRN2 NeuronCores.

Computes out = x @ W_eff^T + bias where
  W_eff = W + sum_f tanh(A_f) @ B_f @ C_f
Column-parallel: W, As, bias sharded along out_features across 8 cores;
x, Bs, Cs replicated. Each core computes its [8192, 512] output shard.

Per-core device algorithm:
  1. tanhA[(f,r), o] = tanh(AsT shard)                        (ScalarE)
  2. ABT[(f,k), o]   = block-diag(Bs)^T @ tanhA      (1 matmul, float32r)
  3. weffT[i, o]     = WT[i, o] + Cs_packed^T @ ABT  (32 matmuls f32r,
                        DVE adds cast the result to bf16)
  4. out[t, o]       = x @ weffT + bias   (bf16 matmuls, fp32 PSUM; 64
                        token strips x 32 accumulating matmuls; bias is
                        added during the PSUM->SBUF eviction)

x is cast to bf16 host-side (halves HBM traffic; PE rate is identical).
Accuracy: bf16 rounding of x and W_eff dominates, ~2.5e-3 relative rms.
"""

import numpy as np
import ml_dtypes

import concourse.mybir as mybir
import concourse.tile as tile
from concourse import bacc
from concourse.bass_utils import run_bass_kernel_spmd

T, I, O, R, F = 8192, 4096, 4096, 64, 2
N_CORES = 8
OS = O // N_CORES      # 512 out features per core
TS = 128               # tokens per strip
IC = I // 128          # 32 contraction chunks
KF = F * R             # 128 packed (factor, rank) contraction for W_eff
WG = 4                 # wt chunks per DMA group (1 MiB per DMA)

F32 = mybir.dt.float32
F32R = mybir.dt.float32r
BF16 = mybir.dt.bfloat16

_CACHE = {}


def _build(n_strips):
    nc = bacc.Bacc()
    xt = nc.declare_dram_parameter("xt", [n_strips, 128, IC, TS], BF16, isOutput=False)
    wt = nc.declare_dram_parameter("wt", [IC // WG, 128, WG, OS], F32, isOutput=False)
    ct = nc.declare_dram_parameter("ct", [KF, IC, 128], F32R, isOutput=False)
    ast = nc.declare_dram_parameter("ast", [KF, OS], F32, isOutput=False)
    bs = nc.declare_dram_parameter("bs", [KF, KF], F32R, isOutput=False)
    bias_full = nc.declare_dram_parameter("bias_full", [128, OS], F32, isOutput=False)
    out = nc.declare_dram_parameter("out", [n_strips * TS, OS], F32, isOutput=True)

    with tile.TileContext(nc) as tc:
        with (
            tc.tile_pool(name="resident", bufs=1) as resident,
            tc.tile_pool(name="prolog", bufs=1) as prolog,
            tc.tile_pool(name="wtp", bufs=4) as wtp,
            tc.tile_pool(name="xp", bufs=4) as xp,
            tc.tile_pool(name="outp", bufs=3) as outp,
            tc.tile_pool(name="psum_abt", bufs=1, space="PSUM") as psum_abt,
            tc.tile_pool(name="psum_w", bufs=2, space="PSUM") as psum_w,
            tc.tile_pool(name="psum_main", bufs=4, space="PSUM") as psum_main,
        ):
            # ---- prologue: weffT = WT + Cs_packed^T @ ABT ----
            # small DMAs first so the tanh -> ABT chain starts ASAP
            ast_sb = prolog.tile([KF, OS], F32)
            nc.sync.dma_start(ast_sb[:], ast[:])
            bs_sb = prolog.tile([KF, KF], F32R)
            nc.sync.dma_start(bs_sb[:], bs[:])
            ct_sb = prolog.tile([KF, IC, 128], F32R)
            nc.sync.dma_start(ct_sb[:], ct[:])

            weff = resident.tile([128, IC, OS], BF16)
            biasf = resident.tile([128, OS], F32)
            nc.sync.dma_start(biasf[:], bias_full[:])

            tanh_sb = prolog.tile([KF, OS], F32R)
            nc.scalar.activation(
                tanh_sb[:], ast_sb[:], mybir.ActivationFunctionType.Tanh
            )
            abt_ps = psum_abt.tile([KF, OS], F32)
            nc.tensor.matmul(
                abt_ps[:], lhsT=bs_sb[:], rhs=tanh_sb[:], start=True, stop=True
            )
            abt = prolog.tile([KF, OS], F32R)
            nc.vector.tensor_copy(out=abt[:], in_=abt_ps[:])
            for g in range(IC // WG):
                wt_sb = wtp.tile([128, WG, OS], F32)
                nc.sync.dma_start(wt_sb[:], wt[g])
                for j in range(WG):
                    ic = g * WG + j
                    wps = psum_w.tile([128, OS], F32)
                    nc.tensor.matmul(
                        wps[:], lhsT=ct_sb[:, ic, :], rhs=abt[:],
                        start=True, stop=True,
                    )
                    nc.vector.tensor_add(
                        out=weff[:, ic, :], in0=wps[:], in1=wt_sb[:, j, :]
                    )

            # ---- main loop: out strips ----
            for tm in range(n_strips):
                x_sb = xp.tile([128, IC, TS], BF16)
                nc.sync.dma_start(x_sb[:], xt[tm])
                ps = psum_main.tile([128, OS], F32)
                for ic in range(IC):
                    nc.tensor.matmul(
                        ps[:],
                        lhsT=x_sb[:, ic, :],
                        rhs=weff[:, ic, :],
                        start=(ic == 0),
                        stop=(ic == IC - 1),
                    )
                o_sb = outp.tile([128, OS], F32)
                nc.vector.tensor_add(out=o_sb[:], in0=ps[:], in1=biasf[:])
                nc.sync.dma_start(out[tm * TS : (tm + 1) * TS, :], o_sb[:])
    nc.compile()
    return nc


def _get_nc(n_strips):
    if n_strips not in _CACHE:
        _CACHE[n_strips] = _build(n_strips)
    return _CACHE[n_strips]


def _prepare_inputs(x, W, bias, As, Bs, Cs, n_strips):
    t_used = n_strips * TS
    # x strips: xt[tm][p, c, n] = x[tm*TS + n, c*128 + p], cast to bf16
    xt = np.ascontiguousarray(
        x[:t_used].reshape(n_strips, TS, IC, 128).transpose(0, 3, 2, 1)
    ).astype(ml_dtypes.bfloat16)
    ct = np.ascontiguousarray(Cs.reshape(KF, IC, 128))
    # block-diagonal packed B: bs[(f,r'), (f,k)] = Bs[f, r', k]
    bs = np.zeros((KF, KF), dtype=np.float32)
    for f in range(F):
        bs[f * R : (f + 1) * R, f * R : (f + 1) * R] = Bs[f]
    in_maps = []
    for c in range(N_CORES):
        Ws = W[c * OS : (c + 1) * OS]                  # [OS, I]
        wtc = np.ascontiguousarray(Ws.T.reshape(IC // WG, WG, 128, OS)
                                   .transpose(0, 2, 1, 3))
        As_c = As[:, c * OS : (c + 1) * OS, :]         # [F, OS, R]
        # packed [(f, r), o]
        astc = np.ascontiguousarray(As_c.transpose(0, 2, 1).reshape(KF, OS))
        bias_fullc = np.ascontiguousarray(
            np.broadcast_to(bias[c * OS : (c + 1) * OS][None, :], (128, OS))
        )
        in_maps.append(
            {"xt": xt, "wt": wtc, "ct": ct, "ast": astc, "bs": bs,
             "bias_full": bias_fullc}
        )
    return in_maps


def _run(x, W, bias, As, Bs, Cs, n_strips, trace=False):
    nc = _get_nc(n_strips)
    in_maps = _prepare_inputs(x, W, bias, As, Bs, Cs, n_strips)
    res = run_bass_kernel_spmd(nc, in_maps, core_ids=list(range(N_CORES)),
                               trace=trace)
    full = np.concatenate([res.results[c]["out"] for c in range(N_CORES)], axis=1)
    return full, res


def kernel(x, W, bias, As, Bs, Cs):
    full, _ = _run(x, W, bias, As, Bs, Cs, T // TS)
    return full


# revision 15
# speedup vs baseline: 1.1917x; 1.1917x over previous
"""ABCLinear distributed Bass kernel for 8 TRN2 NeuronCores.

Computes out = x @ W_eff^T + bias where
  W_eff = W + sum_f tanh(A_f) @ B_f @ C_f
Column-parallel: W, As, bias sharded along out_features across 8 cores;
x, Bs, Cs replicated. Each core computes its [8192, 512] output shard.

Per-core device algorithm:
  1. tanhA[(f,r), o] = tanh(AsT shard)                        (ScalarE)
  2. ABT[(f,k), o]   = block-diag(Bs)^T @ tanhA      (1 matmul, float32r)
  3. weffT[i, o]     = WT[i, o] + Cs_packed^T @ ABT  (32 matmuls f32r,
                        DVE adds cast the result to bf16)
  4. out[t, o]       = x @ weffT + bias   (bf16 matmuls, fp32 PSUM; 64
                        token strips x 32 accumulating matmuls; bias is
                        added during the PSUM->SBUF eviction)

x is cast to bf16 host-side (halves HBM traffic; PE rate is identical).
Accuracy: bf16 rounding of x and W_eff dominates, ~2.5e-3 relative rms.
"""

import numpy as np
import ml_dtypes

import concourse.mybir as mybir
import concourse.tile as tile
from concourse import bacc
from concourse.bass_utils import run_bass_kernel_spmd

T, I, O, R, F = 8192, 4096, 4096, 64, 2
N_CORES = 8
OS = O // N_CORES      # 512 out features per core
TS = 128               # tokens per strip
IC = I // 128          # 32 contraction chunks
KF = F * R             # 128 packed (factor, rank) contraction for W_eff
WG = 4                 # wt chunks per DMA group (1 MiB per DMA)

F32 = mybir.dt.float32
F32R = mybir.dt.float32r
BF16 = mybir.dt.bfloat16

_CACHE = {}


def _build(n_strips, xbufs=4, ct_split=1, wtbufs=8, psbufs=4):
    nc = bacc.Bacc()
    xt = nc.declare_dram_parameter("xt", [n_strips, 128, IC, TS], BF16, isOutput=False)
    wt = nc.declare_dram_parameter("wt", [IC // WG, 128, WG, OS], F32, isOutput=False)
    ct = nc.declare_dram_parameter("ct", [KF, IC, 128], F32R, isOutput=False)
    ast = nc.declare_dram_parameter("ast", [KF, OS], F32, isOutput=False)
    bs = nc.declare_dram_parameter("bs", [KF, KF], F32R, isOutput=False)
    bias_full = nc.declare_dram_parameter("bias_full", [128, OS], F32, isOutput=False)
    out = nc.declare_dram_parameter("out", [n_strips * TS, OS], F32, isOutput=True)

    with tile.TileContext(nc) as tc:
        with (
            tc.tile_pool(name="resident", bufs=1) as resident,
            tc.tile_pool(name="prolog", bufs=1) as prolog,
            tc.tile_pool(name="wtp", bufs=wtbufs) as wtp,
            tc.tile_pool(name="xp", bufs=xbufs) as xp,
            tc.tile_pool(name="outp", bufs=3) as outp,
            tc.tile_pool(name="psum_abt", bufs=1, space="PSUM") as psum_abt,
            tc.tile_pool(name="psum_w", bufs=2, space="PSUM") as psum_w,
            tc.tile_pool(name="psum_main", bufs=psbufs, space="PSUM") as psum_main,
        ):
            # ---- prologue: weffT = WT + Cs_packed^T @ ABT ----
            # small DMAs first so the tanh -> ABT chain starts ASAP
            ast_sb = prolog.tile([KF, OS], F32)
            nc.sync.dma_start(ast_sb[:], ast[:])
            bs_sb = prolog.tile([KF, KF], F32R)
            nc.sync.dma_start(bs_sb[:], bs[:])
            ct_sb = prolog.tile([KF, IC, 128], F32R)
            csz = IC // ct_split
            for g in range(ct_split):
                nc.sync.dma_start(
                    ct_sb[:, g * csz : (g + 1) * csz, :],
                    ct[:, g * csz : (g + 1) * csz, :],
                )

            weff = resident.tile([128, IC, OS], BF16)
            biasf = resident.tile([128, OS], F32)
            nc.sync.dma_start(biasf[:], bias_full[:])

            tanh_sb = prolog.tile([KF, OS], F32R)
            nc.scalar.activation(
                tanh_sb[:], ast_sb[:], mybir.ActivationFunctionType.Tanh
            )
            abt_ps = psum_abt.tile([KF, OS], F32)
            nc.tensor.matmul(
                abt_ps[:], lhsT=bs_sb[:], rhs=tanh_sb[:], start=True, stop=True
            )
            abt = prolog.tile([KF, OS], F32R)
            nc.vector.tensor_copy(out=abt[:], in_=abt_ps[:])
            for g in range(IC // WG):
                wt_sb = wtp.tile([128, WG, OS], F32)
                nc.sync.dma_start(wt_sb[:], wt[g])
                for j in range(WG):
                    ic = g * WG + j
                    wps = psum_w.tile([128, OS], F32)
                    nc.tensor.matmul(
                        wps[:], lhsT=ct_sb[:, ic, :], rhs=abt[:],
                        start=True, stop=True,
                    )
                    nc.vector.tensor_add(
                        out=weff[:, ic, :], in0=wps[:], in1=wt_sb[:, j, :]
                    )

            # ---- main loop: out strips ----
            for tm in range(n_strips):
                x_sb = xp.tile([128, IC, TS], BF16)
                nc.sync.dma_start(x_sb[:], xt[tm])
                ps = psum_main.tile([128, OS], F32)
                for ic in range(IC):
                    nc.tensor.matmul(
                        ps[:],
                        lhsT=x_sb[:, ic, :],
                        rhs=weff[:, ic, :],
                        start=(ic == 0),
                        stop=(ic == IC - 1),
                    )
                o_sb = outp.tile([128, OS], F32)
                nc.vector.tensor_add(out=o_sb[:], in0=ps[:], in1=biasf[:])
                nc.sync.dma_start(out[tm * TS : (tm + 1) * TS, :], o_sb[:])
    nc.compile()
    return nc


def _get_nc(n_strips, **opts):
    key = (n_strips, tuple(sorted(opts.items())))
    if key not in _CACHE:
        _CACHE[key] = _build(n_strips, **opts)
    return _CACHE[key]


def _prepare_inputs(x, W, bias, As, Bs, Cs, n_strips):
    t_used = n_strips * TS
    # x strips: xt[tm][p, c, n] = x[tm*TS + n, c*128 + p], cast to bf16
    xt = np.ascontiguousarray(
        x[:t_used].reshape(n_strips, TS, IC, 128).transpose(0, 3, 2, 1)
    ).astype(ml_dtypes.bfloat16)
    ct = np.ascontiguousarray(Cs.reshape(KF, IC, 128))
    # block-diagonal packed B: bs[(f,r'), (f,k)] = Bs[f, r', k]
    bs = np.zeros((KF, KF), dtype=np.float32)
    for f in range(F):
        bs[f * R : (f + 1) * R, f * R : (f + 1) * R] = Bs[f]
    in_maps = []
    for c in range(N_CORES):
        Ws = W[c * OS : (c + 1) * OS]                  # [OS, I]
        wtc = np.ascontiguousarray(Ws.T.reshape(IC // WG, WG, 128, OS)
                                   .transpose(0, 2, 1, 3))
        As_c = As[:, c * OS : (c + 1) * OS, :]         # [F, OS, R]
        # packed [(f, r), o]
        astc = np.ascontiguousarray(As_c.transpose(0, 2, 1).reshape(KF, OS))
        bias_fullc = np.ascontiguousarray(
            np.broadcast_to(bias[c * OS : (c + 1) * OS][None, :], (128, OS))
        )
        in_maps.append(
            {"xt": xt, "wt": wtc, "ct": ct, "ast": astc, "bs": bs,
             "bias_full": bias_fullc}
        )
    return in_maps


def _run(x, W, bias, As, Bs, Cs, n_strips, trace=False, **opts):
    nc = _get_nc(n_strips, **opts)
    in_maps = _prepare_inputs(x, W, bias, As, Bs, Cs, n_strips)
    res = run_bass_kernel_spmd(nc, in_maps, core_ids=list(range(N_CORES)),
                               trace=trace)
    full = np.concatenate([res.results[c]["out"] for c in range(N_CORES)], axis=1)
    return full, res


def kernel(x, W, bias, As, Bs, Cs):
    full, _ = _run(x, W, bias, As, Bs, Cs, T // TS)
    return full
